# revision 13
# baseline (speedup 1.0000x reference)
"""Trainium2 Bass kernel: grouped similarity-gating normalization.

Reference computation (per batch b, group g, cpg=64 channels, hw=784):
    means[c]  = mean_hw(x[c, :])
    s[hw]     = sum_c x[c, hw] * means[c]
    t         = (s - mean(s)) * rsqrt(var(s) + eps)
    gate      = sigmoid(t * weight[g] + bias[g])
    out[c,hw] = x[c, hw] * gate[hw]

Sharding: data-parallel over batch B=64 across 8 cores (8 batches/core).

Per-core layout: one SBUF tile [128, 4, 786] per batch holds channels
c = 4*p + j (p = partition, j = free chunk); group(c) = c//64 = p//16,
i.e. each group owns a 16-partition band.  All 8 input tiles are DMAd
up front on the sync queue; per-batch pipeline:

  - channel sums: DVE reduce (j0/j1) + ACT copy-accum (j2/j3); all of
    ACT's work stays inside the sigmoid act-table (copy/square/sigmoid)
    so the act table is loaded exactly once for the whole kernel.
  - s via PE: 4 accumulating fp32r matmuls with lhsT[p,q] = means[p]
    masked to the group band (built from a 1/HW-scaled indicator const;
    j0/j1 on DVE, j2/j3 on ACT Copy-with-scale).  Column HW of each xt
    chunk holds -sums/HW so the second matmul chunk also accumulates
    -mu = -mean(s) for free.
  - var via ACT Square(bias=-mu, accum_out); rsqrt(var+eps) on DVE with
    the 0x5f3759df seed + 2 Newton steps, batched over PAIRs of batches
    to amortize the per-instruction overhead of the [128,2] chain.
  - gate = sigmoid(s*a + c) in one activation with per-partition
    scale/bias APs, a = rstd*weight[g], c = bias[g] - mu*a.
  - gating multiply in-place into xt: j0/j1 fused on DVE, j2/j3 fused
    on GpSimd; out-DMA per batch triggered from the sync engine.
"""

import sys

if "/opt/trn_rl_repo" not in sys.path:
    sys.path.insert(0, "/opt/trn_rl_repo")

from contextlib import ExitStack

import numpy as np

import concourse.bacc as bacc
import concourse.bass as bass
import concourse.tile as tile
from concourse import mybir
from concourse.bass_utils import run_bass_kernel_spmd

B, C, H, W = 64, 512, 28, 28
G = 8
HW = H * W          # 784
NCORES = 8
BLOC = B // NCORES  # 8 batches per core
NP = 128            # SBUF partitions
NJ = C // NP        # 4 channel chunks per partition (c = NJ*p + j)
PBAND = NP // G     # 16 partitions per group
EPS = 1e-5
F32 = mybir.dt.float32
F32R = mybir.dt.float32r
MMCHUNK = 512       # max fp32 moving free dim per matmul

_cache: dict = {}

# implementation choices (bisectable)
LHST_ACT = True     # build lhsT j2/j3 on ACT (Copy w/ scale AP) vs DVE
PAIR = 2            # batches per rsqrt-chain group
NR_ITERS = 2        # Newton steps for rsqrt
OUT_TRIG = "sync"   # engine whose queue carries the output DMAs
GATE_BF16 = True    # sigmoid writes bf16 gate (halves gate SBUF traffic)
NACT_COPIES = 2     # channel-sum chunks on ACT copy-accum (rest: DVE reduce)
N_DVE_MUL = 1       # gating-mul chunks on DVE (rest fused on GpSimd)


def _emit(tc, nc, xs, m8h, wv, bv, ys):
    AF = mybir.ActivationFunctionType
    OP = mybir.AluOpType
    I32 = mybir.dt.int32
    NPAIR = BLOC // PAIR
    with ExitStack() as ctx:
        consts = ctx.enter_context(tc.tile_pool(name="consts", bufs=1))
        xpool = ctx.enter_context(tc.tile_pool(name="xpool", bufs=BLOC))
        mpool = ctx.enter_context(tc.tile_pool(name="mpool", bufs=3))
        vpool = ctx.enter_context(tc.tile_pool(name="vpool", bufs=3))
        gpool = ctx.enter_context(tc.tile_pool(name="gpool", bufs=3))
        spsum = ctx.enter_context(tc.tile_pool(name="spsum", bufs=3, space="PSUM"))

        # write-only sink for copy-accum / square-accum primary outputs:
        # keep it in PSUM so the dead writes stay off the SBUF ports
        # (SBUF bandwidth is the contended resource: DMA in+out, PE reads,
        # and three vector-ish engines all stream it concurrently)
        dummy = spsum.tile([NP, HW], F32, bufs=1)

        # m8h carries the [NP, NP] block-banded indicator scaled by 1/HW:
        # m8h[p, q] = (p//PBAND == q//PBAND) / HW; wv/bv are 16x-replicated
        m16h_sb = consts.tile([NP, NP], F32)
        nc.sync.dma_start(out=m16h_sb[:], in_=m8h[:])
        wv_sb = consts.tile([NP, 1], F32)
        nc.sync.dma_start(out=wv_sb[:], in_=wv[:])
        bv_sb = consts.tile([NP, 1], F32)
        nc.sync.dma_start(out=bv_sb[:], in_=bv[:])

        xts = {}
        sums_t = {}
        lhsts = {}
        pss = {}
        nmus = {}
        hvs = {}

        xf = lambda ap: ap.bitcast(F32)

        def dma_in(b):
            # column HW of each chunk later holds -sums/HW so the matmul's
            # second chunk accumulates -mu (2 cols: fp32r needs even widths)
            xt = xpool.tile([NP, NJ, HW + 2], F32R)
            nc.sync.dma_start(out=xt[:, 0:2, 0:HW], in_=xs[b, :, 0:2, :])
            nc.sync.dma_start(out=xt[:, 2:4, 0:HW], in_=xs[b, :, 2:4, :])
            xts[b] = xt

        def phase1(b):
            # channel sums, mu columns, banded lhsT
            xt = xts[b]
            sums = mpool.tile([NP, NJ], F32, tag="sums")
            ndve = NJ - NACT_COPIES
            nc.vector.reduce_sum(
                out=sums[:, 0:ndve], in_=xf(xt[:, 0:ndve, 0:HW]),
                axis=mybir.AxisListType.X,
            )
            for j in range(ndve, NJ):
                nc.scalar.activation(
                    out=dummy[:], in_=xf(xt[:, j, 0:HW]), func=AF.Copy,
                    accum_out=sums[:, j : j + 1],
                )
            # ps[:, HW] accumulates sum_c means_c * (-sums_c/HW) = -mu
            nc.vector.tensor_scalar_mul(
                xt[:, :, HW : HW + 2],
                sums[:].unsqueeze(2).to_broadcast([NP, NJ, 2]),
                -1.0 / HW,
            )
            lhsT = mpool.tile([NP, NJ, NP], F32R, tag="lhsT")
            for j in range(NJ):
                if LHST_ACT and j >= 2:
                    # same-engine dep: read-accum for sums[:, j] precedes this
                    nc.scalar.activation(
                        out=lhsT[:, j, :], in_=m16h_sb[:], func=AF.Copy,
                        scale=sums[:, j : j + 1],
                    )
                else:
                    nc.vector.tensor_scalar(
                        out=lhsT[:, j, :], in0=m16h_sb[:],
                        scalar1=sums[:, j : j + 1], scalar2=None, op0=OP.mult,
                    )
            sums_t[b] = sums
            lhsts[b] = lhsT

        def phase2(b):
            # s (replicated per 16-band) in cols 0:HW; -mu in col HW
            xt = xts[b]
            lhsT = lhsts.pop(b)
            ps = spsum.tile([NP, HW + 2], F32)
            for c0 in (0, MMCHUNK):
                c1 = min(c0 + MMCHUNK, HW + 2)
                for j in range(NJ):
                    nc.tensor.matmul(
                        ps[:, c0:c1], lhsT[:, j, :], xt[:, j, c0:c1],
                        start=(j == 0), stop=(j == NJ - 1),
                    )
            pss[b] = ps

        def stats(b):
            # -mu to SBUF (ACT scale/bias APs must live in SBUF) + variance
            ps = pss[b]
            k, i = divmod(b, PAIR)
            if i == 0:
                hvs[k] = vpool.tile([NP, PAIR], F32, tag="hv", name="hv")
            nmu = vpool.tile([NP, 1], F32, tag=f"nmu{i}")
            nc.vector.tensor_copy(nmu[:], ps[:, HW : HW + 1])
            nmus[b] = nmu
            nc.scalar.activation(
                out=dummy[:], in_=ps[:, 0:HW], func=AF.Square, bias=nmu[:],
                accum_out=hvs[k][:, i : i + 1],
            )

        def chain(k):
            # a = w * sqrt(HW) * rsqrt(hwvar + HW*eps); c = b + (-mu)*a
            # (magic-seed + Newton on DVE, batched over the pair)
            hv = hvs.pop(k)
            u = vpool.tile([NP, PAIR], F32, tag="u")
            nc.vector.tensor_scalar_add(u[:], hv[:], HW * EPS)
            y0 = vpool.tile([NP, PAIR], I32, tag="y0")
            nc.vector.tensor_scalar(
                out=y0[:], in0=u[:].bitcast(I32), scalar1=1, scalar2=None,
                op0=OP.arith_shift_right,
            )
            nc.vector.tensor_scalar(
                out=y0[:], in0=y0[:], scalar1=0xFFFFFFFF, scalar2=None,
                op0=OP.bitwise_xor,
            )
            nc.vector.tensor_scalar(
                out=y0[:], in0=y0[:], scalar1=0x5F3759E0, scalar2=None, op0=OP.add
            )
            yc = y0[:].bitcast(F32)
            for it in range(NR_ITERS):
                p_t = vpool.tile([NP, PAIR], F32, tag=f"p{it}")
                nc.vector.tensor_mul(p_t[:], yc, yc)
                m_t = vpool.tile([NP, PAIR], F32, tag=f"m{it}")
                nc.vector.scalar_tensor_tensor(
                    out=m_t[:], in0=u[:], scalar=-0.5, in1=p_t[:],
                    op0=OP.mult, op1=OP.mult,
                )
                y_t = vpool.tile([NP, PAIR], F32, tag=f"y{it}")
                nc.vector.scalar_tensor_tensor(
                    out=y_t[:], in0=m_t[:], scalar=1.5, in1=yc,
                    op0=OP.add, op1=OP.mult,
                )
                yc = y_t[:]
            a2 = vpool.tile([NP, PAIR], F32, tag="a2")
            nc.vector.tensor_scalar(
                out=a2[:], in0=yc, scalar1=wv_sb[:], scalar2=float(np.sqrt(HW)),
                op0=OP.mult, op1=OP.mult,
            )
            c2 = vpool.tile([NP, PAIR], F32, tag="c2")
            for i in range(PAIR):
                nc.vector.scalar_tensor_tensor(
                    out=c2[:, i : i + 1], in0=nmus.pop(k * PAIR + i)[:],
                    scalar=a2[:, i : i + 1], in1=bv_sb[:],
                    op0=OP.mult, op1=OP.add,
                )
            return a2, c2

        def gating(b, a2, c2):
            # sigmoid gate + in-place gating multiply + output DMA
            i = b % PAIR
            ps = pss.pop(b)
            gate = gpool.tile([NP, HW], mybir.dt.bfloat16 if GATE_BF16 else F32,
                              tag="gate")
            nc.scalar.activation(
                out=gate[:], in_=ps[:, 0:HW], func=AF.Sigmoid,
                scale=a2[:, i : i + 1], bias=c2[:, i : i + 1],
            )
            xt = xts.pop(b)
            nd = N_DVE_MUL
            gbd = gate[:].unsqueeze(1).to_broadcast([NP, nd, HW])
            nc.vector.tensor_mul(xt[:, 0:nd, 0:HW], xf(xt[:, 0:nd, 0:HW]), gbd)
            gbg = gate[:].unsqueeze(1).to_broadcast([NP, NJ - nd, HW])
            nc.gpsimd.tensor_mul(xt[:, nd:NJ, 0:HW], xf(xt[:, nd:NJ, 0:HW]), gbg)
            getattr(nc, OUT_TRIG).dma_start(out=ys[b], in_=xf(xt[:, :, 0:HW]))

        # all inputs up front on the sync queue; outputs enqueue behind them
        for b in range(BLOC):
            dma_in(b)
        phase1(0)
        phase1(1)
        phase2(0)
        phase2(1)
        for k in range(NPAIR):
            b0, b1 = 2 * k, 2 * k + 1
            if k + 1 < NPAIR:
                phase1(b0 + 2)
                phase1(b1 + 2)
            stats(b0)
            stats(b1)
            if k + 1 < NPAIR:
                phase2(b0 + 2)
                phase2(b1 + 2)
            a2, c2 = chain(k)
            gating(b0, a2, c2)
            gating(b1, a2, c2)


def _build_nc():
    nc = bacc.Bacc("TRN2", debug=False)
    xs = nc.dram_tensor("xs", [BLOC, NP, NJ, HW], F32R, kind="ExternalInput")
    m8h = nc.dram_tensor("m8h", [NP, NP], F32, kind="ExternalInput")
    wv = nc.dram_tensor("wv", [NP, 1], F32, kind="ExternalInput")
    bv = nc.dram_tensor("bv", [NP, 1], F32, kind="ExternalInput")
    ys = nc.dram_tensor("ys", [BLOC, NP, NJ, HW], F32, kind="ExternalOutput")
    with tile.TileContext(nc) as tc:
        _emit(tc, nc, xs, m8h, wv, bv, ys)
    nc.compile()
    return nc


def get_nc():
    if "nc" not in _cache:
        _cache["nc"] = _build_nc()
    return _cache["nc"]


def make_in_maps(x, weight, bias):
    x = np.ascontiguousarray(np.asarray(x, dtype=np.float32))
    weight = np.asarray(weight, dtype=np.float32).reshape(G)
    bias = np.asarray(bias, dtype=np.float32).reshape(G)
    # [core, b, p, j, hw] with c = NJ*p + j
    xs = x.reshape(NCORES, BLOC, NP, NJ, HW)
    band = np.arange(NP) // PBAND
    m8h = (band[:, None] == band[None, :]).astype(np.float32) / HW
    wv = np.ascontiguousarray(np.repeat(weight, PBAND)[:, None])
    bv = np.ascontiguousarray(np.repeat(bias, PBAND)[:, None])
    return [
        {"xs": np.ascontiguousarray(xs[i]), "m8h": m8h, "wv": wv, "bv": bv}
        for i in range(NCORES)
    ]


def run(x, weight, bias, trace=False, **spmd_kwargs):
    nc = get_nc()
    in_maps = make_in_maps(x, weight, bias)
    res = run_bass_kernel_spmd(
        nc, in_maps, core_ids=list(range(NCORES)), trace=trace, **spmd_kwargs
    )
    out = np.stack([res.results[i]["ys"] for i in range(NCORES)])
    return out.reshape(B, C, H, W), res


def kernel(x, weight, bias, groups=G, **_ignored):
    assert int(groups) == G
    out, _ = run(x, weight, bias, trace=False)
    return out
